# revision 1
# baseline (speedup 1.0000x reference)
"""Trainium2 Bass kernel for nn_ChannelAttentionModule (B=8, H=W=128, C=512).

Reference computation (per sample, q = inputs reshaped to [HW, C] = [16384, 512]):
    S = q^T @ q                      # [C, C]
    P = softmax(max_row(S) - S)      # == softmax(-S) row-wise (shift invariant)
    out = gamma * (q @ P) + q

Numerical scheme: q is split as q ~= hi + lo (both fp16, lo^T lo dropped,
~1e-7 relative).  With X = hi^T lo (note lo^T hi == X^T exactly):
    S = hi^T hi + X + X^T = A + A^T
where A = strict_upper(hi^T hi) + 0.5 * blockdiag(hi^T hi) + X is accumulated
in 4 PSUM banks (block-row per bank), and S = A + A^T is assembled once on
the small [512, 512] matrix via fp32 PE transposes.  The value pass uses
M = gamma * softmax(-S) + I so the gamma-scale and residual add ride through
the matmul.

Sharding: data-parallel over batch, one sample per NeuronCore, 8 cores, no
cross-core communication (gamma replicated host-side).

Per-core schedule:
  pass 1: stream 64 x [128, 2, 512] fp32 slabs of q (512KB contiguous DMAs);
          cast to resident fp16 hi (VectorE), lo = q - hi, hi/2 (exact);
          accumulate A on TensorE (fp16, 3328 PE columns per 128-row chunk).
  fixup:  S = A + A^T; row-min + exp with fused row-sum (ScalarE);
          M = gamma*P + I cast to fp16.
  pass 2: per 128-row chunk: transpose resident hi via matmul-vs-identity,
          4 value matmuls accumulate out = q @ M in PSUM, evacuate
          (VectorE/ScalarE alternating), 512KB DMAs back to HBM.
"""

import sys

for _p in ("/opt/trn_rl_repo",):
    if _p not in sys.path:
        sys.path.insert(0, _p)

from contextlib import ExitStack

import numpy as np

import concourse.bass as bass
import concourse.mybir as mybir
import concourse.tile as tile
from concourse import bacc

F32 = mybir.dt.float32
F16 = mybir.dt.float16

P = 128
C = 512
MB = C // P  # 4 c-blocks

B, H, W = 8, 128, 128
N_ROWS_FULL = H * W  # 16384 rows per sample
N_CORES = 8


def build(n_rows=N_ROWS_FULL, s_mode="f16hl", dma_tr=False):
    """Per-core kernel. Inputs: x [n_rows, C] f32, gammab [128,1] f32,
    ident [128,128] f16. Output: out [n_rows, C] f32."""
    assert n_rows % 256 == 0
    nsup = n_rows // 256
    nsub = n_rows // 128
    use_lo = s_mode == "f16hl"

    nc = bacc.Bacc(trn_type="TRN2", name="chanattn")
    x = nc.dram_tensor("x", [n_rows, C], F32, kind="ExternalInput")
    gb = nc.dram_tensor("gammab", [P, 1], F32, kind="ExternalInput")
    idm = nc.dram_tensor("ident", [P, P], F16, kind="ExternalInput")
    out = nc.dram_tensor("out", [n_rows, C], F32, kind="ExternalOutput")

    # row r = 256*i + 2*p + j: partition p reads 4KB contiguous per super-chunk
    xv = x[:].rearrange("(i p j) c -> i p j c", p=P, j=2)
    ov = out[:].rearrange("(i p j) c -> i p j c", p=P, j=2)

    with tile.TileContext(nc) as tc, ExitStack() as ctx:
        pers = ctx.enter_context(tc.tile_pool(name="pers", bufs=1))
        p_in = ctx.enter_context(tc.tile_pool(name="p_in", bufs=3))
        p_h = ctx.enter_context(tc.tile_pool(name="p_h", bufs=2))
        p_sm = ctx.enter_context(tc.tile_pool(name="p_sm", bufs=2))
        p_qc = ctx.enter_context(tc.tile_pool(name="p_qc", bufs=10))
        p_out = ctx.enter_context(tc.tile_pool(name="p_out", bufs=2))
        ps_a_ctx = ExitStack()
        ps_a = ps_a_ctx.enter_context(tc.tile_pool(name="ps_a", bufs=1, space="PSUM"))

        # resident fp16 hi = round(q), laid out [p, sub, c]
        hi_res = pers.tile([P, nsub, C], F16)
        xf0 = p_in.tile([P, 2, C], F32, tag="xf", name="xf")
        nc.sync.dma_start(xf0[:], xv[0])
        ident16 = pers.tile([P, P], F16)
        nc.sync.dma_start(ident16[:], idm[:])
        gamma_sb = pers.tile([P, 1], F32)
        nc.sync.dma_start(gamma_sb[:], gb[:])

        # A = strict_upper(hh) + 0.5*diag-blocks(hh) + X, block-row m per bank
        a_ps = [ps_a.tile([P, C], F32, tag=f"a{m}", name=f"a{m}") for m in range(MB)]

        # ---- pass 1 ----
        for i in range(nsup):
            if i == 0:
                xf = xf0
            else:
                xf = p_in.tile([P, 2, C], F32, tag="xf", name="xf")
                nc.sync.dma_start(xf[:], xv[i])
            hi2 = hi_res[:, 2 * i : 2 * i + 2, :]
            nc.vector.tensor_copy(hi2, xf[:])
            xh2 = p_h.tile([P, 2, C], F16, tag="xh2", name="xh2")
            nc.scalar.mul(xh2[:], hi2, 0.5)
            if use_lo:
                xl = p_h.tile([P, 2, C], F16, tag="xl", name="xl")
                nc.vector.tensor_tensor(xl[:], xf[:], hi2, mybir.AluOpType.subtract)
            for j in range(2):
                s = 2 * i + j
                first = s == 0
                last = s == nsub - 1
                hs = hi_res[:, s, :]
                for m in range(MB):
                    lhs = hs[:, m * P : (m + 1) * P]
                    # 0.5 * hh diagonal block. Only THIS matmul at s==0 may
                    # carry start=True: start clears has_written for the
                    # whole bank, so later same-bank groups must rely on the
                    # cleared bits (overwrite-then-set) instead of start.
                    nc.tensor.matmul(
                        a_ps[m][:, m * P : (m + 1) * P],
                        lhs,
                        xh2[:, j, m * P : (m + 1) * P],
                        start=first,
                        stop=last,
                        skip_group_check=True,
                    )
                    # strict-upper hh blocks
                    if m < MB - 1:
                        nc.tensor.matmul(
                            a_ps[m][:, (m + 1) * P :],
                            lhs,
                            hs[:, (m + 1) * P :],
                            start=False,
                            stop=last,
                            skip_group_check=True,
                        )
                    if use_lo:
                        nc.tensor.matmul(
                            a_ps[m][:],
                            lhs,
                            xl[:, j, :],
                            start=False,
                            stop=last,
                            skip_group_check=True,
                        )

        # ---- assemble S = A + A^T in SBUF ----
        # A^T via hi/lo fp16 split + regular matmul-vs-identity transposes
        # (fp32 PE transposes are fused-LDW and can only carry one sync wait,
        # which walrus rejects here).  hi+lo carries ~21 bits of A; the lost
        # precision only affects entries with |A| huge (the diag blocks),
        # which are irrelevant to the row-min softmax.
        s_sb = pers.tile([P, MB, C], F32)
        for m in range(MB):
            nc.vector.tensor_copy(s_sb[:, m, :], a_ps[m][:])
        ps_a_ctx.close()
        ps_t = ctx.enter_context(tc.tile_pool(name="ps_t", bufs=3, space="PSUM"))
        ps_v = ctx.enter_context(tc.tile_pool(name="ps_v", bufs=3, space="PSUM"))
        ah = pers.tile([P, MB, C], F16)
        al = pers.tile([P, MB, C], F16)
        for m in range(MB):
            nc.vector.tensor_copy(ah[:, m, :], s_sb[:, m, :])
            nc.vector.tensor_tensor(
                al[:, m, :], s_sb[:, m, :], ah[:, m, :], mybir.AluOpType.subtract
            )
        # S[m, mp] = A[m, mp] + T(Ah[mp, m]) + T(Al[mp, m]); the hi and lo
        # transposes accumulate in PSUM so one DVE add per block suffices.
        for mp in range(MB):
            for m in range(MB):
                tp = ps_t.tile([P, MB, P], F32, tag="tp", name="tp")
                nc.tensor.matmul(
                    tp[:, 0, :],
                    ah[:, mp, m * P : (m + 1) * P],
                    ident16[:],
                    start=True,
                    stop=False,
                )
                nc.tensor.matmul(
                    tp[:, 0, :],
                    al[:, mp, m * P : (m + 1) * P],
                    ident16[:],
                    start=False,
                    stop=True,
                )
                nc.vector.tensor_tensor(
                    s_sb[:, m, mp * P : (mp + 1) * P],
                    s_sb[:, m, mp * P : (mp + 1) * P],
                    tp[:, 0, :],
                    mybir.AluOpType.add,
                )

        # ---- softmax: M = gamma * softmax(-S) + I (fp16) ----
        mfull = pers.tile([P, MB, C], F16)
        for m in range(MB):
            mn = p_sm.tile([P, 1], F32, tag="mn", name="mn")
            nc.vector.tensor_reduce(
                mn[:], s_sb[:, m, :], axis=mybir.AxisListType.X, op=mybir.AluOpType.min
            )
            e = p_sm.tile([P, C], F32, tag="e", name="e")
            z = p_sm.tile([P, 1], F32, tag="z", name="z")
            nc.scalar.activation(
                e[:],
                s_sb[:, m, :],
                mybir.ActivationFunctionType.Exp,
                bias=mn[:],
                scale=-1.0,
                accum_out=z[:],
            )
            rz = p_sm.tile([P, 1], F32, tag="rz", name="rz")
            nc.vector.reciprocal(rz[:], z[:])
            rzg = p_sm.tile([P, 1], F32, tag="rzg", name="rzg")
            nc.vector.tensor_mul(rzg[:], rz[:], gamma_sb[:])
            nc.vector.tensor_scalar_mul(mfull[:, m, :], e[:], rzg[:])
            nc.vector.tensor_tensor(
                mfull[:, m, m * P : (m + 1) * P],
                mfull[:, m, m * P : (m + 1) * P],
                ident16[:],
                mybir.AluOpType.add,
            )

        # ---- pass 2: out = q @ M (chunk-transpose + 4 accumulating matmuls) ----
        qc_tiles = {}

        def emit_tr(s):
            qc = p_qc.tile([P, MB, P], F16, tag="qc", name="qc")
            if dma_tr:
                for m in range(MB):
                    nc.sync.dma_start_transpose(
                        qc[:, m, :], hi_res[:, s, m * P : (m + 1) * P]
                    )
            else:
                tp = ps_t.tile([P, MB, P], F32, tag="tp", name="tp")
                for m in range(MB):
                    nc.tensor.matmul(
                        tp[:, m, :],
                        hi_res[:, s, m * P : (m + 1) * P],
                        ident16[:],
                        start=True,
                        stop=True,
                    )
                nc.scalar.copy(qc[:], tp[:])
            qc_tiles[s] = qc

        for s0 in range(min(10, nsub)):
            emit_tr(s0)
        for i in range(nsup):
            of = p_out.tile([P, 2, C], F32, tag="of", name="of")
            for j in range(2):
                s = 2 * i + j
                if s + 10 < nsub:
                    emit_tr(s + 10)
                vp = ps_v.tile([P, C], F32, tag="vp", name="vp")
                qc = qc_tiles.pop(s)
                for m in range(MB):
                    nc.tensor.matmul(
                        vp[:],
                        qc[:, m, :],
                        mfull[:, m, :],
                        start=(m == 0),
                        stop=(m == MB - 1),
                    )
                if j == 0:
                    nc.vector.tensor_copy(of[:, j, :], vp[:])
                else:
                    nc.scalar.copy(of[:, j, :], vp[:])
            nc.sync.dma_start(ov[i], of[:])

    nc.compile()
    return nc


def make_in_map(x_sample, gamma):
    return {
        "x": np.ascontiguousarray(x_sample, dtype=np.float32),
        "gammab": np.full((P, 1), gamma, dtype=np.float32),
        "ident": np.eye(P, dtype=np.float16),
    }


_NC_CACHE = {}


def _get_nc(n_rows=N_ROWS_FULL, s_mode="f16hl"):
    key = (n_rows, s_mode)
    if key not in _NC_CACHE:
        _NC_CACHE[key] = build(n_rows, s_mode)
    return _NC_CACHE[key]


def kernel(inputs, gamma):
    from concourse.bass_utils import run_bass_kernel_spmd

    x = np.asarray(inputs, dtype=np.float32)
    g = float(np.asarray(gamma, dtype=np.float32))
    assert x.shape == (B, H, W, C), x.shape

    nc = _get_nc()
    in_maps = [make_in_map(x[b].reshape(N_ROWS_FULL, C), g) for b in range(B)]
    res = run_bass_kernel_spmd(nc, in_maps, core_ids=list(range(N_CORES)))
    out = np.stack([r["out"] for r in res.results], axis=0)
    return out.reshape(B, H, W, C).astype(np.float32)



# revision 2
# speedup vs baseline: 1.2674x; 1.2674x over previous
"""Trainium2 Bass kernel for nn_ChannelAttentionModule (B=8, H=W=128, C=512).

Reference computation (per sample, q = inputs reshaped to [HW, C] = [16384, 512]):
    S = q^T @ q                      # [C, C]
    P = softmax(max_row(S) - S)      # == softmax(-S) row-wise (shift invariant)
    out = gamma * (q @ P) + q

Numerical scheme: S is computed directly in float32r (TF32-like, 11 mantissa
bits, full fp16 matmul throughput at moving-dim >= 256).  Validated against
the exact harness inputs: worst output rel err 7.0e-3 (gate 2e-2).  Only the
upper triangle of S is computed (block-row m covers column blocks >= m, with
block-row 3 widened to cols [256:512] to stay >= 256 wide); the lower blocks
are reconstructed by PE transposes of the upper blocks (fp16 hi/lo split of
the fp32 PSUM values).  The value pass uses M = gamma * softmax(-S) + I in
fp16 so the gamma-scale and residual add ride through the matmul.

Sharding: data-parallel over batch, one sample per NeuronCore, 8 cores, no
cross-core communication (gamma replicated host-side).

Per-core schedule:
  pass 1: stream 64 x [128, 2, 512] fp32 slabs of q (512KB contiguous DMAs,
          alternating between the SP and Activation HWDGE queues); round to
          resident-free fp32r (DVE) and cast fp16 hi resident (DVE/ScalarE
          split); 4 accumulating fp32r matmuls per 128-row chunk.
  fixup:  S = A + A^T via 5 off-diag block transposes; row-min + exp with
          fused row-sum (ScalarE); M = gamma*P + I cast to fp16.  First
          pass-2 chunk transposes are emitted here to keep the PE warm.
  pass 2: per 128-row chunk: transpose resident hi via matmul-vs-identity,
          4 value matmuls accumulate out = q @ M in PSUM, evacuate
          (VectorE/ScalarE alternating), 512KB DMAs back to HBM on
          alternating queues.
"""

import sys

for _p in ("/opt/trn_rl_repo",):
    if _p not in sys.path:
        sys.path.insert(0, _p)

from contextlib import ExitStack

import numpy as np

import concourse.bass as bass
import concourse.mybir as mybir
import concourse.tile as tile
from concourse import bacc

F32 = mybir.dt.float32
F32R = mybir.dt.float32r
F16 = mybir.dt.float16

P = 128
C = 512
MB = C // P  # 4 c-blocks

B, H, W = 8, 128, 128
N_ROWS_FULL = H * W  # 16384 rows per sample
N_CORES = 8


def build(n_rows=N_ROWS_FULL):
    """Per-core kernel. Inputs: x [n_rows, C] f32, gammab [128,1] f32,
    ident [128,128] f16. Output: out [n_rows, C] f32."""
    assert n_rows % 256 == 0
    nsup = n_rows // 256
    nsub = n_rows // 128

    nc = bacc.Bacc(trn_type="TRN2", name="chanattn")
    x = nc.dram_tensor("x", [n_rows, C], F32, kind="ExternalInput")
    gb = nc.dram_tensor("gammab", [P, 1], F32, kind="ExternalInput")
    idm = nc.dram_tensor("ident", [P, P], F16, kind="ExternalInput")
    out = nc.dram_tensor("out", [n_rows, C], F32, kind="ExternalOutput")

    # row r = 256*i + 2*p + j: partition p reads 4KB contiguous per super-chunk
    xv = x[:].rearrange("(i p j) c -> i p j c", p=P, j=2)
    ov = out[:].rearrange("(i p j) c -> i p j c", p=P, j=2)

    with tile.TileContext(nc) as tc, ExitStack() as ctx:
        pers = ctx.enter_context(tc.tile_pool(name="pers", bufs=1))
        p_in = ctx.enter_context(tc.tile_pool(name="p_in", bufs=3))
        p_xr = ctx.enter_context(tc.tile_pool(name="p_xr", bufs=2))
        p_sm = ctx.enter_context(tc.tile_pool(name="p_sm", bufs=2))
        p_qc = ctx.enter_context(tc.tile_pool(name="p_qc", bufs=8))
        p_out = ctx.enter_context(tc.tile_pool(name="p_out", bufs=3))
        ps_a_ctx = ExitStack()
        ps_a = ps_a_ctx.enter_context(tc.tile_pool(name="ps_a", bufs=1, space="PSUM"))

        # resident fp16 hi = round(q), laid out [p, sub, c]
        hi_res = pers.tile([P, nsub, C], F16)
        xf0 = p_in.tile([P, 2, C], F32, tag="xf", name="xf")
        nc.sync.dma_start(xf0[:], xv[0])
        ident16 = pers.tile([P, P], F16)
        nc.sync.dma_start(ident16[:], idm[:])
        gamma_sb = pers.tile([P, 1], F32)
        nc.sync.dma_start(gamma_sb[:], gb[:])

        # A = upper(S) in fp32r, block-row m per bank; bank 3 widened to
        # cols [256:512] so every fp32r matmul keeps moving-dim >= 256
        # (below 256 fp32r drops to 1/4 throughput).
        a_ps = [ps_a.tile([P, C], F32, tag=f"a{m}", name=f"a{m}") for m in range(MB)]
        acols = [slice(0, C), slice(P, C), slice(2 * P, C), slice(2 * P, C)]

        # ---- pass 1 ----
        for i in range(nsup):
            if i == 0:
                xf = xf0
            else:
                xf = p_in.tile([P, 2, C], F32, tag="xf", name="xf")
                if i % 2 == 0:
                    nc.sync.dma_start(xf[:], xv[i])
                else:
                    nc.scalar.dma_start(xf[:], xv[i])
            xr = p_xr.tile([P, 2, C], F32R, tag="xr", name="xr")
            nc.vector.tensor_copy(xr[:], xf[:])
            hi2 = hi_res[:, 2 * i : 2 * i + 2, :]
            nc.vector.tensor_copy(hi2[:, 0, :], xf[:, 0, :])
            nc.scalar.copy(hi2[:, 1, :], xf[:, 1, :])
            for j in range(2):
                s = 2 * i + j
                first = s == 0
                last = s == nsub - 1
                for m in range(MB):
                    nc.tensor.matmul(
                        a_ps[m][:, acols[m]],
                        xr[:, j, m * P : (m + 1) * P],
                        xr[:, j, acols[m]],
                        start=first,
                        stop=last,
                        skip_group_check=True,
                    )

        # ---- assemble S = A + A^T in SBUF ----
        s_sb = pers.tile([P, MB, C], F32)
        nc.vector.tensor_copy(s_sb[:, 0, :], a_ps[0][:])
        nc.scalar.copy(s_sb[:, 1, P:], a_ps[1][:, P:])
        nc.vector.tensor_copy(s_sb[:, 2, 2 * P :], a_ps[2][:, 2 * P :])
        nc.scalar.copy(s_sb[:, 3, 2 * P :], a_ps[3][:, 2 * P :])
        ps_a_ctx.close()
        ps_t = ctx.enter_context(tc.tile_pool(name="ps_t", bufs=3, space="PSUM"))
        ps_v = ctx.enter_context(tc.tile_pool(name="ps_v", bufs=4, space="PSUM"))

        # lower blocks (b, a) = T(upper block (a, b)) for the 5 upper blocks
        # not covered by bank 3's widened row; fp16 hi/lo keeps ~21 bits.
        TRB = [(0, 1), (0, 2), (0, 3), (1, 2), (1, 3)]
        ah = pers.tile([P, len(TRB), P], F16)
        al = pers.tile([P, len(TRB), P], F16)
        for k, (a, b) in enumerate(TRB):
            src = s_sb[:, a, b * P : (b + 1) * P]
            nc.vector.tensor_copy(ah[:, k, :], src)
            nc.vector.tensor_tensor(
                al[:, k, :], src, ah[:, k, :], mybir.AluOpType.subtract
            )
        tfix = [
            ps_t.tile([P, MB, P], F32, tag="tp", name="tfix0"),
            ps_t.tile([P, MB, P], F32, tag="tp", name="tfix1"),
        ]
        for k, (a, b) in enumerate(TRB):
            tt = tfix[k // MB][:, k % MB, :]
            nc.tensor.matmul(tt, ah[:, k, :], ident16[:], start=True, stop=False)
            nc.tensor.matmul(tt, al[:, k, :], ident16[:], start=False, stop=True)
            nc.scalar.copy(s_sb[:, b, a * P : (a + 1) * P], tt)

        # ---- softmax: M = gamma * softmax(-S) + I (fp16) ----
        mfull = pers.tile([P, MB, C], F16)
        for m in range(MB):
            mn = p_sm.tile([P, 1], F32, tag="mn", name="mn")
            nc.vector.tensor_reduce(
                mn[:], s_sb[:, m, :], axis=mybir.AxisListType.X, op=mybir.AluOpType.min
            )
            e = p_sm.tile([P, C], F32, tag="e", name="e")
            z = p_sm.tile([P, 1], F32, tag="z", name="z")
            nc.scalar.activation(
                e[:],
                s_sb[:, m, :],
                mybir.ActivationFunctionType.Exp,
                bias=mn[:],
                scale=-1.0,
                accum_out=z[:],
            )
            rz = p_sm.tile([P, 1], F32, tag="rz", name="rz")
            nc.vector.reciprocal(rz[:], z[:])
            rzg = p_sm.tile([P, 1], F32, tag="rzg", name="rzg")
            nc.vector.tensor_mul(rzg[:], rz[:], gamma_sb[:])
            nc.vector.tensor_scalar_mul(mfull[:, m, :], e[:], rzg[:])
            nc.vector.tensor_tensor(
                mfull[:, m, m * P : (m + 1) * P],
                mfull[:, m, m * P : (m + 1) * P],
                ident16[:],
                mybir.AluOpType.add,
            )

        # ---- pass 2: out = q @ M (chunk-transpose + 4 accumulating matmuls) ----
        qc_tiles = {}

        def emit_tr(s):
            qc = p_qc.tile([P, MB, P], F16, tag="qc", name="qc")
            tp = ps_t.tile([P, MB, P], F32, tag="tp", name="tp")
            for m in range(MB):
                nc.tensor.matmul(
                    tp[:, m, :],
                    hi_res[:, s, m * P : (m + 1) * P],
                    ident16[:],
                    start=True,
                    stop=True,
                )
            nc.vector.tensor_copy(qc[:], tp[:])
            qc_tiles[s] = qc

        LA = 8
        for s0 in range(min(LA, nsub)):
            emit_tr(s0)
        for i in range(nsup):
            of = p_out.tile([P, 2, C], F32, tag="of", name="of")
            for j in range(2):
                s = 2 * i + j
                if s + LA < nsub:
                    emit_tr(s + LA)
                vp = ps_v.tile([P, C], F32, tag="vp", name="vp")
                qc = qc_tiles.pop(s)
                for m in range(MB):
                    nc.tensor.matmul(
                        vp[:],
                        qc[:, m, :],
                        mfull[:, m, :],
                        start=(m == 0),
                        stop=(m == MB - 1),
                    )
                if j == 0:
                    nc.vector.tensor_copy(of[:, j, :], vp[:])
                else:
                    nc.scalar.copy(of[:, j, :], vp[:])
            if i % 2 == 0:
                nc.sync.dma_start(ov[i], of[:])
            else:
                nc.scalar.dma_start(ov[i], of[:])

    nc.compile()
    return nc


def make_in_map(x_sample, gamma):
    return {
        "x": np.ascontiguousarray(x_sample, dtype=np.float32),
        "gammab": np.full((P, 1), gamma, dtype=np.float32),
        "ident": np.eye(P, dtype=np.float16),
    }


_NC_CACHE = {}


def _get_nc(n_rows=N_ROWS_FULL):
    key = n_rows
    if key not in _NC_CACHE:
        _NC_CACHE[key] = build(n_rows)
    return _NC_CACHE[key]


def kernel(inputs, gamma):
    from concourse.bass_utils import run_bass_kernel_spmd

    x = np.asarray(inputs, dtype=np.float32)
    g = float(np.asarray(gamma, dtype=np.float32))
    assert x.shape == (B, H, W, C), x.shape

    nc = _get_nc()
    in_maps = [make_in_map(x[b].reshape(N_ROWS_FULL, C), g) for b in range(B)]
    res = run_bass_kernel_spmd(nc, in_maps, core_ids=list(range(N_CORES)))
    out = np.stack([r["out"] for r in res.results], axis=0)
    return out.reshape(B, H, W, C).astype(np.float32)


# revision 8
# speedup vs baseline: 1.3518x; 1.0666x over previous
"""Trainium2 Bass kernel for nn_ChannelAttentionModule (B=8, H=W=128, C=512).

Reference computation (per sample, q = inputs reshaped to [HW, C] = [16384, 512]):
    S = q^T @ q                      # [C, C]
    P = softmax(max_row(S) - S)      # == softmax(-S) row-wise (shift invariant)
    out = gamma * (q @ P) + q

Numerical scheme: S is computed directly in float32r (TF32-like, 11 mantissa
bits, full fp16 matmul throughput at moving-dim >= 256).  Validated against
the exact harness inputs: worst output rel err 7.0e-3 (gate 2e-2).  Only the
upper triangle of S is computed (block-row m covers column blocks >= m, with
block-row 3 widened to cols [256:512] to stay >= 256 wide); the lower blocks
are reconstructed by PE transposes of the upper blocks (fp16 hi/lo split of
the fp32 PSUM values).  The value pass uses M = gamma * softmax(-S) + I in
fp16 so the gamma-scale and residual add ride through the matmul.

Sharding: data-parallel over batch, one sample per NeuronCore, 8 cores, no
cross-core communication (gamma replicated host-side).

Per-core schedule:
  pass 1: stream 64 x [128, 2, 512] fp32 slabs of q (512KB contiguous DMAs,
          alternating between the SP and Activation HWDGE queues); round to
          resident-free fp32r (DVE) and cast fp16 hi resident (DVE/ScalarE
          split); 4 accumulating fp32r matmuls per 128-row chunk.
  fixup:  S = A + A^T via 5 off-diag block transposes; row-min + exp with
          fused row-sum (ScalarE); M = gamma*P + I cast to fp16.  First
          pass-2 chunk transposes are emitted here to keep the PE warm.
  pass 2: per 128-row chunk: transpose resident hi via matmul-vs-identity,
          4 value matmuls accumulate out = q @ M in PSUM, evacuate
          (VectorE/ScalarE alternating), 512KB DMAs back to HBM on
          alternating queues.
"""

import sys

for _p in ("/opt/trn_rl_repo",):
    if _p not in sys.path:
        sys.path.insert(0, _p)

from contextlib import ExitStack

import numpy as np

import concourse.bass as bass
import concourse.mybir as mybir
import concourse.tile as tile
from concourse import bacc

F32 = mybir.dt.float32
F32R = mybir.dt.float32r
F16 = mybir.dt.float16

P = 128
C = 512
MB = C // P  # 4 c-blocks

B, H, W = 8, 128, 128
N_ROWS_FULL = H * W  # 16384 rows per sample
N_CORES = 8


def build(n_rows=N_ROWS_FULL, J=4):
    """Per-core kernel. Inputs: x [n_rows, C] f32, gammab [128,1] f32,
    ident [128,128] f16. Output: out [n_rows, C] f32."""
    assert n_rows % (P * J) == 0
    nsup = n_rows // (P * J)
    nsub = n_rows // 128

    nc = bacc.Bacc(trn_type="TRN2", name="chanattn")
    x = nc.dram_tensor("x", [n_rows, C], F32, kind="ExternalInput")
    gb = nc.dram_tensor("gammab", [P, 1], F32, kind="ExternalInput")
    idm = nc.dram_tensor("ident", [P, P], F16, kind="ExternalInput")
    out = nc.dram_tensor("out", [n_rows, C], F32, kind="ExternalOutput")

    # row r = P*J*i + J*p + j: partition p reads 2KB*J contiguous per
    # super-chunk -> 8KB descriptors / 1MB transfers at J=4 (bigger
    # descriptors lift the HBM DMA ceiling well above the ~235 GB/s seen
    # with 4KB descriptors).
    xv = x[:].rearrange("(i p j) c -> i p j c", p=P, j=J)
    ov = out[:].rearrange("(i p j) c -> i p j c", p=P, j=J)

    with tile.TileContext(nc) as tc, ExitStack() as ctx:
        pers = ctx.enter_context(tc.tile_pool(name="pers", bufs=1))
        p_in = ctx.enter_context(tc.tile_pool(name="p_in", bufs=2))
        p_xr = ctx.enter_context(tc.tile_pool(name="p_xr", bufs=2))
        p_sm = ctx.enter_context(tc.tile_pool(name="p_sm", bufs=2))
        p_qc = ctx.enter_context(tc.tile_pool(name="p_qc", bufs=8))
        p_out = ctx.enter_context(tc.tile_pool(name="p_out", bufs=2))
        ps_a_ctx = ExitStack()
        ps_a = ps_a_ctx.enter_context(tc.tile_pool(name="ps_a", bufs=1, space="PSUM"))

        # resident fp16 hi = round(q), laid out [p, sub, c]
        hi_res = pers.tile([P, nsub, C], F16)
        xf0 = p_in.tile([P, J, C], F32, tag="xf", name="xf")
        nc.sync.dma_start(xf0[:], xv[0])
        ident16 = pers.tile([P, P], F16)
        nc.sync.dma_start(ident16[:], idm[:])
        gamma_sb = pers.tile([P, 1], F32)
        nc.sync.dma_start(gamma_sb[:], gb[:])

        # A = upper(S) in fp32r, block-row m per bank; bank 3 widened to
        # cols [256:512] so every fp32r matmul keeps moving-dim >= 256
        # (below 256 fp32r drops to 1/4 throughput).
        a_ps = [ps_a.tile([P, C], F32, tag=f"a{m}", name=f"a{m}") for m in range(MB)]
        acols = [slice(0, C), slice(P, C), slice(2 * P, C), slice(2 * P, C)]

        # ---- pass 1 ----
        for i in range(nsup):
            if i == 0:
                xf = xf0
            else:
                xf = p_in.tile([P, J, C], F32, tag="xf", name="xf")
                if i % 2 == 0:
                    nc.sync.dma_start(xf[:], xv[i])
                else:
                    nc.scalar.dma_start(xf[:], xv[i])
            xr = p_xr.tile([P, J, C], F32R, tag="xr", name="xr")
            nc.vector.tensor_copy(xr[:], xf[:])
            hij = hi_res[:, J * i : J * i + J, :]
            for j in range(J):
                if j % 2 == 0:
                    nc.vector.tensor_copy(hij[:, j, :], xf[:, j, :])
                else:
                    nc.scalar.copy(hij[:, j, :], xf[:, j, :])
            for j in range(J):
                s = J * i + j
                first = s == 0
                last = s == nsub - 1
                for m in range(MB):
                    nc.tensor.matmul(
                        a_ps[m][:, acols[m]],
                        xr[:, j, m * P : (m + 1) * P],
                        xr[:, j, acols[m]],
                        start=first,
                        stop=last,
                        skip_group_check=True,
                    )

        # ---- assemble S = A + A^T in SBUF ----
        s_sb = pers.tile([P, MB, C], F32)
        nc.vector.tensor_copy(s_sb[:, 0, :], a_ps[0][:])
        nc.scalar.copy(s_sb[:, 1, P:], a_ps[1][:, P:])
        nc.vector.tensor_copy(s_sb[:, 2, 2 * P :], a_ps[2][:, 2 * P :])
        nc.scalar.copy(s_sb[:, 3, 2 * P :], a_ps[3][:, 2 * P :])
        ps_a_ctx.close()
        ps_t = ctx.enter_context(tc.tile_pool(name="ps_t", bufs=3, space="PSUM"))
        ps_v = ctx.enter_context(tc.tile_pool(name="ps_v", bufs=4, space="PSUM"))

        # lower blocks (b, a) = T(upper block (a, b)) for the 5 upper blocks
        # not covered by bank 3's widened row; fp16 hi/lo keeps ~21 bits.
        TRB = [(0, 1), (0, 2), (0, 3), (1, 2), (1, 3)]
        ah = pers.tile([P, len(TRB), P], F16)
        al = pers.tile([P, len(TRB), P], F16)
        for k, (a, b) in enumerate(TRB):
            src = s_sb[:, a, b * P : (b + 1) * P]
            nc.vector.tensor_copy(ah[:, k, :], src)
            nc.vector.tensor_tensor(
                al[:, k, :], src, ah[:, k, :], mybir.AluOpType.subtract
            )
        tfix = [
            ps_t.tile([P, MB, P], F32, tag="tp", name="tfix0"),
            ps_t.tile([P, MB, P], F32, tag="tp", name="tfix1"),
        ]
        for k, (a, b) in enumerate(TRB):
            tt = tfix[k // MB][:, k % MB, :]
            nc.tensor.matmul(tt, ah[:, k, :], ident16[:], start=True, stop=False)
            nc.tensor.matmul(tt, al[:, k, :], ident16[:], start=False, stop=True)
            nc.scalar.copy(s_sb[:, b, a * P : (a + 1) * P], tt)

        # ---- softmax: M = gamma * softmax(-S) + I (fp16) ----
        mfull = pers.tile([P, MB, C], F16)
        for m in range(MB):
            mn = p_sm.tile([P, 1], F32, tag="mn", name="mn")
            nc.vector.tensor_reduce(
                mn[:], s_sb[:, m, :], axis=mybir.AxisListType.X, op=mybir.AluOpType.min
            )
            e = p_sm.tile([P, C], F32, tag="e", name="e")
            z = p_sm.tile([P, 1], F32, tag="z", name="z")
            nc.scalar.activation(
                e[:],
                s_sb[:, m, :],
                mybir.ActivationFunctionType.Exp,
                bias=mn[:],
                scale=-1.0,
                accum_out=z[:],
            )
            rz = p_sm.tile([P, 1], F32, tag="rz", name="rz")
            nc.vector.reciprocal(rz[:], z[:])
            rzg = p_sm.tile([P, 1], F32, tag="rzg", name="rzg")
            nc.vector.tensor_mul(rzg[:], rz[:], gamma_sb[:])
            nc.vector.tensor_scalar_mul(mfull[:, m, :], e[:], rzg[:])
            nc.vector.tensor_tensor(
                mfull[:, m, m * P : (m + 1) * P],
                mfull[:, m, m * P : (m + 1) * P],
                ident16[:],
                mybir.AluOpType.add,
            )

        # ---- pass 2: out = q @ M (chunk-transpose + 4 accumulating matmuls) ----
        qc_tiles = {}

        def emit_tr(s):
            qc = p_qc.tile([P, MB, P], F16, tag="qc", name="qc")
            tp = ps_t.tile([P, MB, P], F32, tag="tp", name="tp")
            for m in range(MB):
                nc.tensor.matmul(
                    tp[:, m, :],
                    hi_res[:, s, m * P : (m + 1) * P],
                    ident16[:],
                    start=True,
                    stop=True,
                )
            nc.vector.tensor_copy(qc[:], tp[:])
            qc_tiles[s] = qc

        LA = 8
        for s0 in range(min(LA, nsub)):
            emit_tr(s0)
        for i in range(nsup):
            of = p_out.tile([P, J, C], F32, tag="of", name="of")
            for j in range(J):
                s = J * i + j
                if s + LA < nsub:
                    emit_tr(s + LA)
                vp = ps_v.tile([P, C], F32, tag="vp", name="vp")
                qc = qc_tiles.pop(s)
                for m in range(MB):
                    nc.tensor.matmul(
                        vp[:],
                        qc[:, m, :],
                        mfull[:, m, :],
                        start=(m == 0),
                        stop=(m == MB - 1),
                    )
                if j % 2 == 0:
                    nc.vector.tensor_copy(of[:, j, :], vp[:])
                else:
                    nc.scalar.copy(of[:, j, :], vp[:])
            if i % 2 == 0:
                nc.sync.dma_start(ov[i], of[:])
            else:
                nc.scalar.dma_start(ov[i], of[:])

    nc.compile()
    return nc


def make_in_map(x_sample, gamma):
    return {
        "x": np.ascontiguousarray(x_sample, dtype=np.float32),
        "gammab": np.full((P, 1), gamma, dtype=np.float32),
        "ident": np.eye(P, dtype=np.float16),
    }


_NC_CACHE = {}


def _get_nc(n_rows=N_ROWS_FULL):
    key = n_rows
    if key not in _NC_CACHE:
        _NC_CACHE[key] = build(n_rows)
    return _NC_CACHE[key]


def kernel(inputs, gamma):
    from concourse.bass_utils import run_bass_kernel_spmd

    x = np.asarray(inputs, dtype=np.float32)
    g = float(np.asarray(gamma, dtype=np.float32))
    assert x.shape == (B, H, W, C), x.shape

    nc = _get_nc()
    in_maps = [make_in_map(x[b].reshape(N_ROWS_FULL, C), g) for b in range(B)]
    res = run_bass_kernel_spmd(nc, in_maps, core_ids=list(range(N_CORES)))
    out = np.stack([r["out"] for r in res.results], axis=0)
    return out.reshape(B, H, W, C).astype(np.float32)


# revision 17
# speedup vs baseline: 1.3836x; 1.0235x over previous
"""Trainium2 Bass kernel for nn_ChannelAttentionModule (B=8, H=W=128, C=512).

Reference computation (per sample, q = inputs reshaped to [HW, C] = [16384, 512]):
    S = q^T @ q                      # [C, C]
    P = softmax(max_row(S) - S)      # == softmax(-S) row-wise (shift invariant)
    out = gamma * (q @ P) + q

Numerical scheme: S is computed directly in float32r (TF32-like, 11 mantissa
bits, full fp16 matmul throughput at moving-dim >= 256).  Validated against
the exact harness inputs: worst output rel err 7.0e-3 (gate 2e-2).  Only the
upper triangle of S is computed (block-row m covers column blocks >= m, with
block-row 3 widened to cols [256:512] to stay >= 256 wide); the lower blocks
are reconstructed by PE transposes of the upper blocks (fp16 hi/lo split of
the fp32 PSUM values).  The value pass uses M = gamma * softmax(-S) + I in
fp16 so the gamma-scale and residual add ride through the matmul.

Sharding: data-parallel over batch, one sample per NeuronCore, 8 cores, no
cross-core communication (gamma replicated host-side).

Per-core schedule:
  pass 1: stream 64 x [128, 2, 512] fp32 slabs of q (512KB contiguous DMAs,
          alternating between the SP and Activation HWDGE queues); round to
          resident-free fp32r (DVE) and cast fp16 hi resident (DVE/ScalarE
          split); 4 accumulating fp32r matmuls per 128-row chunk.
  fixup:  S = A + A^T via 5 off-diag block transposes; row-min + exp with
          fused row-sum (ScalarE); M = gamma*P + I cast to fp16.  First
          pass-2 chunk transposes are emitted here to keep the PE warm.
  pass 2: per 128-row chunk: transpose resident hi via matmul-vs-identity,
          4 value matmuls accumulate out = q @ M in PSUM, evacuate
          (VectorE/ScalarE alternating), 512KB DMAs back to HBM on
          alternating queues.
"""

import sys

for _p in ("/opt/trn_rl_repo",):
    if _p not in sys.path:
        sys.path.insert(0, _p)

from contextlib import ExitStack

import numpy as np

import concourse.bass as bass
import concourse.mybir as mybir
import concourse.tile as tile
from concourse import bacc

F32 = mybir.dt.float32
F32R = mybir.dt.float32r
F16 = mybir.dt.float16

P = 128
C = 512
MB = C // P  # 4 c-blocks

B, H, W = 8, 128, 128
N_ROWS_FULL = H * W  # 16384 rows per sample
N_CORES = 8


def build(n_rows=N_ROWS_FULL, J=4):
    """Per-core kernel. Inputs: x [n_rows, C] f32, gammab [128,1] f32,
    ident [128,128] f16. Output: out [n_rows, C] f32."""
    assert n_rows % (P * J) == 0
    nsup = n_rows // (P * J)
    nsub = n_rows // 128

    nc = bacc.Bacc(trn_type="TRN2", name="chanattn")
    x = nc.dram_tensor("x", [n_rows, C], F32, kind="ExternalInput")
    gb = nc.dram_tensor("gammab", [P, 1], F32, kind="ExternalInput")
    idm = nc.dram_tensor("ident", [P, P], F16, kind="ExternalInput")
    out = nc.dram_tensor("out", [n_rows, C], F32, kind="ExternalOutput")

    # row r = P*J*i + J*p + j: partition p reads 2KB*J contiguous per
    # super-chunk -> 8KB descriptors / 1MB transfers at J=4 (bigger
    # descriptors lift the HBM DMA ceiling well above the ~235 GB/s seen
    # with 4KB descriptors).
    xv = x[:].rearrange("(i p j) c -> i p j c", p=P, j=J)
    ov = out[:].rearrange("(i p j) c -> i p j c", p=P, j=J)

    with tile.TileContext(nc) as tc, ExitStack() as ctx:
        pers = ctx.enter_context(tc.tile_pool(name="pers", bufs=1))
        p_in = ctx.enter_context(tc.tile_pool(name="p_in", bufs=3))
        p_xr = ctx.enter_context(tc.tile_pool(name="p_xr", bufs=2))
        p_sm = ctx.enter_context(tc.tile_pool(name="p_sm", bufs=2))
        p_qc = ctx.enter_context(tc.tile_pool(name="p_qc", bufs=8))
        p_out = ctx.enter_context(tc.tile_pool(name="p_out", bufs=2))
        ps_a_ctx = ExitStack()
        ps_a = ps_a_ctx.enter_context(tc.tile_pool(name="ps_a", bufs=1, space="PSUM"))

        # resident fp16 hi = round(q), laid out [p, sub, c]
        hi_res = pers.tile([P, nsub, C], F16)
        ident16 = pers.tile([P, P], F16)
        nc.sync.dma_start(ident16[:], idm[:])
        gamma_sb = pers.tile([P, 1], F32)
        nc.sync.dma_start(gamma_sb[:], gb[:])

        # A = upper(S) in fp32r, block-row m per bank; bank 3 widened to
        # cols [256:512] so every fp32r matmul keeps moving-dim >= 256
        # (below 256 fp32r drops to 1/4 throughput).
        a_ps = [ps_a.tile([P, C], F32, tag=f"a{m}", name=f"a{m}") for m in range(MB)]
        acols = [slice(0, C), slice(P, C), slice(2 * P, C), slice(2 * P, C)]

        # ---- pass 1 ----
        # fp32r rounding in half-super granularity: smaller tiles free SBUF
        # for a 3-deep input prefetch and let each super's first matmuls
        # start after only half the round work.
        def chunk_mms(xr, jj, s):
            first = s == 0
            last = s == nsub - 1
            for m in range(MB):
                nc.tensor.matmul(
                    a_ps[m][:, acols[m]],
                    xr[:, jj, m * P : (m + 1) * P],
                    xr[:, jj, acols[m]],
                    start=first,
                    stop=last,
                    skip_group_check=True,
                )

        for i in range(nsup):
            xf = p_in.tile([P, J, C], F32, tag="xf", name="xf")
            if i % 2 == 0:
                nc.sync.dma_start(xf[:], xv[i])
            else:
                nc.scalar.dma_start(xf[:], xv[i])
            hij = hi_res[:, J * i : J * i + J, :]
            for h in range(2):
                xr = p_xr.tile([P, 2, C], F32R, tag="xr", name="xr")
                nc.vector.tensor_copy(xr[:], xf[:, 2 * h : 2 * h + 2, :])
                for jj in range(2):
                    j = 2 * h + jj
                    if j % 2 == 0:
                        nc.vector.tensor_copy(hij[:, j, :], xf[:, j, :])
                    else:
                        nc.scalar.copy(hij[:, j, :], xf[:, j, :])
                    chunk_mms(xr, jj, J * i + j)

        # ---- assemble S = A + A^T in SBUF ----
        s_sb = pers.tile([P, MB, C], F32)
        nc.vector.tensor_copy(s_sb[:, 0, :], a_ps[0][:])
        nc.scalar.copy(s_sb[:, 1, P:], a_ps[1][:, P:])
        nc.vector.tensor_copy(s_sb[:, 2, 2 * P :], a_ps[2][:, 2 * P :])
        nc.scalar.copy(s_sb[:, 3, 2 * P :], a_ps[3][:, 2 * P :])
        ps_a_ctx.close()
        ps_t = ctx.enter_context(tc.tile_pool(name="ps_t", bufs=3, space="PSUM"))
        ps_v = ctx.enter_context(tc.tile_pool(name="ps_v", bufs=4, space="PSUM"))

        # lower blocks (b, a) = T(upper block (a, b)) for the 5 upper blocks
        # not covered by bank 3's widened row; fp16 hi/lo keeps ~21 bits.
        TRB = [(0, 1), (0, 2), (0, 3), (1, 2), (1, 3)]
        ah = pers.tile([P, len(TRB), P], F16)
        al = pers.tile([P, len(TRB), P], F16)
        for k, (a, b) in enumerate(TRB):
            src = s_sb[:, a, b * P : (b + 1) * P]
            nc.vector.tensor_copy(ah[:, k, :], src)
            nc.vector.tensor_tensor(
                al[:, k, :], src, ah[:, k, :], mybir.AluOpType.subtract
            )
        tfix = [
            ps_t.tile([P, MB, P], F32, tag="tp", name="tfix0"),
            ps_t.tile([P, MB, P], F32, tag="tp", name="tfix1"),
        ]
        for k, (a, b) in enumerate(TRB):
            tt = tfix[k // MB][:, k % MB, :]
            nc.tensor.matmul(tt, ah[:, k, :], ident16[:], start=True, stop=False)
            nc.tensor.matmul(tt, al[:, k, :], ident16[:], start=False, stop=True)
            nc.scalar.copy(s_sb[:, b, a * P : (a + 1) * P], tt)

        # ---- softmax: M = gamma * softmax(-S) + I (fp16) ----
        mfull = pers.tile([P, MB, C], F16)
        for m in range(MB):
            mn = p_sm.tile([P, 1], F32, tag="mn", name="mn")
            nc.vector.tensor_reduce(
                mn[:], s_sb[:, m, :], axis=mybir.AxisListType.X, op=mybir.AluOpType.min
            )
            e = p_sm.tile([P, C], F32, tag="e", name="e")
            z = p_sm.tile([P, 1], F32, tag="z", name="z")
            nc.scalar.activation(
                e[:],
                s_sb[:, m, :],
                mybir.ActivationFunctionType.Exp,
                bias=mn[:],
                scale=-1.0,
                accum_out=z[:],
            )
            rz = p_sm.tile([P, 1], F32, tag="rz", name="rz")
            nc.vector.reciprocal(rz[:], z[:])
            rzg = p_sm.tile([P, 1], F32, tag="rzg", name="rzg")
            nc.vector.tensor_mul(rzg[:], rz[:], gamma_sb[:])
            nc.vector.tensor_scalar_mul(mfull[:, m, :], e[:], rzg[:])
            nc.vector.tensor_tensor(
                mfull[:, m, m * P : (m + 1) * P],
                mfull[:, m, m * P : (m + 1) * P],
                ident16[:],
                mybir.AluOpType.add,
            )

        # ---- pass 2: out = q @ M (chunk-transpose + 4 accumulating matmuls) ----
        qc_tiles = {}

        def emit_tr(s):
            qc = p_qc.tile([P, MB, P], F16, tag="qc", name="qc")
            tp = ps_t.tile([P, MB, P], F32, tag="tp", name="tp")
            for m in range(MB):
                nc.tensor.matmul(
                    tp[:, m, :],
                    hi_res[:, s, m * P : (m + 1) * P],
                    ident16[:],
                    start=True,
                    stop=True,
                )
            nc.vector.tensor_copy(qc[:], tp[:])
            qc_tiles[s] = qc

        LA = 8
        for s0 in range(min(LA, nsub)):
            emit_tr(s0)
        for i in range(nsup):
            of = p_out.tile([P, J, C], F32, tag="of", name="of")
            for j in range(J):
                s = J * i + j
                if s + LA < nsub:
                    emit_tr(s + LA)
                vp = ps_v.tile([P, C], F32, tag="vp", name="vp")
                qc = qc_tiles.pop(s)
                for m in range(MB):
                    nc.tensor.matmul(
                        vp[:],
                        qc[:, m, :],
                        mfull[:, m, :],
                        start=(m == 0),
                        stop=(m == MB - 1),
                    )
                if j % 2 == 0:
                    nc.vector.tensor_copy(of[:, j, :], vp[:])
                else:
                    nc.scalar.copy(of[:, j, :], vp[:])
            if i == nsup - 1:
                # split the final transfer across both queues to shorten the tail
                nc.sync.dma_start(ov[i][:, 0 : J // 2, :], of[:, 0 : J // 2, :])
                nc.scalar.dma_start(ov[i][:, J // 2 :, :], of[:, J // 2 :, :])
            elif i % 2 == 0:
                nc.sync.dma_start(ov[i], of[:])
            else:
                nc.scalar.dma_start(ov[i], of[:])

    nc.compile()
    return nc


def make_in_map(x_sample, gamma):
    return {
        "x": np.ascontiguousarray(x_sample, dtype=np.float32),
        "gammab": np.full((P, 1), gamma, dtype=np.float32),
        "ident": np.eye(P, dtype=np.float16),
    }


_NC_CACHE = {}


def _get_nc(n_rows=N_ROWS_FULL):
    key = n_rows
    if key not in _NC_CACHE:
        _NC_CACHE[key] = build(n_rows)
    return _NC_CACHE[key]


def kernel(inputs, gamma):
    from concourse.bass_utils import run_bass_kernel_spmd

    x = np.asarray(inputs, dtype=np.float32)
    g = float(np.asarray(gamma, dtype=np.float32))
    assert x.shape == (B, H, W, C), x.shape

    nc = _get_nc()
    in_maps = [make_in_map(x[b].reshape(N_ROWS_FULL, C), g) for b in range(B)]
    res = run_bass_kernel_spmd(nc, in_maps, core_ids=list(range(N_CORES)))
    out = np.stack([r["out"] for r in res.results], axis=0)
    return out.reshape(B, H, W, C).astype(np.float32)
